# revision 1
# baseline (speedup 1.0000x reference)
# Trainium2 Bass kernel for nn_AStarPathfinder.
#
# Pipeline (per core j of NC, SPMD identical instruction stream):
#   - host passes a channel-major zero-padded feature slice for a column band
#   - device computes sobel/avg-pool stencils in layout A ([C partitions, pixels free]),
#     channel contractions via stationary-operand PE matmuls (outputs pixel-partitioned),
#     MLP via w1-stationary matmuls, cost grid via shifted dot products,
#   - 256 Jacobi Bellman-Ford sweeps, column-sharded with ghost zones:
#     one packed 9-candidate tensor_tensor add + one segmented reduce-min per sweep,
#     partition(row)-shifts via PE shift-matmuls, halo refresh via AllGather every T sweeps.
import numpy as np

BIGF = 1e9
DIRS = [(-1, -1), (-1, 0), (-1, 1), (0, -1), (0, 1), (1, -1), (1, 0), (1, 1)]
F32 = np.float32


def make_cfg(H=256, W=256, C=128, NC=8, K=256, T=8):
    assert C == 128
    cfg = {}
    cfg["H"], cfg["W"], cfg["C"], cfg["NC"], cfg["K"], cfg["T"] = H, W, C, NC, K, T
    cfg["HP"] = H // 2          # partitions for fields / BF (x folded in 2 blocks)
    cfg["XH"] = 2               # number of x blocks
    cfg["BAND"] = W // NC       # owned cols per core
    assert cfg["BAND"] >= 2 * T, "halo exchange strips must fit in owned band"
    cfg["HELD"] = cfg["BAND"] + 2 * T
    cfg["SLICE_W"] = cfg["HELD"] + 4
    cfg["FW"] = cfg["HELD"] + 2  # cost domain cols (held +- 1)
    cfg["XP"] = H + 2
    cfg["FA_FREE"] = cfg["XP"] * cfg["SLICE_W"]
    cfg["CR"] = min(64, cfg["HP"])  # stationary-chunk rows (PE quadrant-aligned)
    assert H % cfg["CR"] == 0
    cfg["NCHK"] = H // cfg["CR"]
    cfg["YB"] = cfg["HELD"] + 2  # BF per-block slot span
    cfg["YLEN"] = 2 * cfg["YB"]
    cfg["SW"] = 2 * cfg["YB"] + 8  # BF slot stride
    cfg["XPS"] = H + 4
    stg_raw = (cfg["FW"] + 4) * cfg["XPS"]
    cfg["STG"] = ((stg_raw + cfg["HP"] - 1) // cfg["HP"]) * cfg["HP"]  # staging floats
    return cfg


# ---------------------------------------------------------------- program ---

def build_program(cfg):
    import concourse.bass as bass
    import concourse.bacc as bacc
    import concourse.mybir as mybir
    import concourse.tile as tile
    from concourse.ap import AP

    H, W, C, NC, K, T = (cfg[k] for k in ("H", "W", "C", "NC", "K", "T"))
    HP, XH, BAND, HELD = (cfg[k] for k in ("HP", "XH", "BAND", "HELD"))
    SLICE_W, FW, XP, YB, YLEN, SW = (
        cfg[k] for k in ("SLICE_W", "FW", "XP", "YB", "YLEN", "SW"))
    FA_FREE, STG, XPS = cfg["FA_FREE"], cfg["STG"], cfg["XPS"]
    CR, NCHK = cfg["CR"], cfg["NCHK"]
    dt = mybir.dt.float32
    Alu = mybir.AluOpType
    Act = mybir.ActivationFunctionType

    nc = bacc.Bacc("TRN2", target_bir_lowering=False, debug=False, num_devices=NC)

    # ---- external inputs (per core) ----
    fa_in = nc.dram_tensor("fa", [C, FA_FREE], dt, kind="ExternalInput")
    w1_in = nc.dram_tensor("w1", [C, 32], dt, kind="ExternalInput")
    b1_in = nc.dram_tensor("b1", [32, 1], dt, kind="ExternalInput")
    w2_in = nc.dram_tensor("w2", [32, 1], dt, kind="ExternalInput")
    elf_in = nc.dram_tensor("elf", [64, 1], dt, kind="ExternalInput")
    hsc_in = nc.dram_tensor("hsc", [HP, 8], dt, kind="ExternalInput")  # d,g,b,info,b2,...
    msk_in = nc.dram_tensor("msk", [HP, 4 * FW * XH], dt, kind="ExternalInput")
    dinit_in = nc.dram_tensor("dinit", [HP, SW], dt, kind="ExternalInput")
    shm_in = nc.dram_tensor("shm", [HP, 4 * HP], dt, kind="ExternalInput")  # su|useam|sd|dseam
    out_t = nc.dram_tensor("out", [HP, BAND * XH * 10], dt, kind="ExternalOutput")
    dbg_names = []
    dbg_t = {}
    if cfg.get("debug_fields"):
        dbg_names = ["d_nall", "d_rs", "d_dot0", "d_dot1", "d_cost0", "d_cost1",
                     "d_geo", "d_wl", "d_rsh0", "d_rsh1", "d_rsh2", "d_rsh3",
                     "e_nall", "e_dot1", "e_cost1", "e_rs"]
        for n in dbg_names:
            dbg_t[n] = nc.dram_tensor(n, [cfg["HP"], cfg["FW"] * 2], dt,
                                      kind="ExternalOutput")
        dbg_t["d_fa"] = nc.dram_tensor("d_fa", [C, cfg["FA_FREE"]], dt,
                                       kind="ExternalOutput")
        dbg_names.append("d_fa")

    # ---- internal DRAM ----
    stg = [nc.dram_tensor(f"stg{i}", [1, STG], dt) for i in range(4)]
    stg_rs = nc.dram_tensor("stg_rs", [1, STG], dt)
    bounce_in = nc.dram_tensor("bounce_in", [HP, 4 * T], dt)
    bounce_out = nc.dram_tensor("bounce_out", [NC * HP, 4 * T], dt, addr_space="Shared")

    CHR = (CR + 2)  # x-chunk temp rows incl halo

    def fap(t, off, dims):
        """AP over tile/dram handle t with explicit free dims (list of [step,count])."""
        base = t[:] if not isinstance(t, AP) else t
        return AP(base.tensor, base.offset + off, [base.ap[0]] + dims)

    with tile.TileContext(nc) as tc:
        with tc.tile_pool(name="persist", bufs=1) as pp, \
             tc.tile_pool(name="chunk", bufs=1) as cp, \
             tc.tile_pool(name="small", bufs=1) as sp, \
             tc.tile_pool(name="psum", bufs=1, space="PSUM") as psp, \
             tc.tile_pool(name="psbf", bufs=2, space="PSUM") as psbf:

            # ---------------- load ----------------
            fA = pp.tile([C, FA_FREE], dt, tag="fA")
            nc.sync.dma_start(fA[:], fa_in[:])
            w1t = sp.tile([C, 32], dt, tag="w1")
            nc.sync.dma_start(w1t[:], w1_in[:])
            b1t = sp.tile([32, 1], dt, tag="b1")
            nc.sync.dma_start(b1t[:], b1_in[:])
            w2t = sp.tile([32, 1], dt, tag="w2")
            nc.sync.dma_start(w2t[:], w2_in[:])
            elft = sp.tile([64, 1], dt, tag="elf")
            nc.sync.dma_start(elft[:], elf_in[:])
            hsct = sp.tile([HP, 8], dt, tag="hsc")
            nc.sync.dma_start(hsct[:], hsc_in[:])
            mskt = sp.tile([HP, 4 * FW * XH], dt, tag="msk")
            nc.sync.dma_start(mskt[:], msk_in[:])
            shmt = sp.tile([HP, 4 * HP], dt, tag="shm")
            nc.sync.dma_start(shmt[:], shm_in[:])

            # reduce vectors: 0: 1/C, 1: ones, 2: ones_lo(0:64), 3: ones_hi(64:128)
            vec = sp.tile([C, 4], dt, tag="vec")
            nc.vector.memset(vec[:, 0:1], 1.0 / C)
            nc.vector.memset(vec[:, 1:2], 1.0)
            nc.vector.memset(vec[:, 2:4], 0.0)
            nc.vector.memset(vec[0:64, 2:3], 1.0)
            nc.vector.memset(vec[64:128, 3:4], 1.0)

            # ---------------- per-pixel field tiles [HP, FW*XH] ----------------
            FXX = FW * XH
            geo_f = sp.tile([HP, FXX], dt, tag="geo")
            nall_f = sp.tile([HP, FXX], dt, tag="nall")
            var_f = sp.tile([HP, FXX], dt, tag="var")
            asq_f = sp.tile([HP, FXX], dt, tag="asq")
            wl_f = sp.tile([HP, FXX], dt, tag="wl")
            dot_f = []
            for i in range(4):
                dti = sp.tile([HP, FXX], dt, tag=f"dot{i}")
                dot_f.append(dti)

            # psum accumulators for stationary reduces packed in one tile;
            # sub-views: one column per (c, xh), 9 groups at stride FXX
            ps_all = psp.tile([HP, 9 * FXX + 16], dt, tag="psall")
            ps_fence = fap(ps_all, 9 * FXX, [[1, 16]])

            # Bootstrap LDWEIGHTS fences for constants (see fence_mm below):
            # f0's weight read may see garbage (harmless, scratch output); its
            # moving-operand wait orders all later weight loads after the vec
            # memsets; f_w1/f_shm then cover the DMA'd constants.
            from concourse.bass import _add_dep_helper as _adh

            def boot_fence(lhsT, rhs_t, n, step):
                outap = fap(ps_all[0:1, :], 9 * FXX, [[1, n]])
                return nc.tensor.matmul(outap, lhsT,
                                        fap(rhs_t[:], 0, [[step, n]]),
                                        start=True, stop=True).ins

            f0 = boot_fence(vec[:, 0:1], vec, 4, 1)
            f_w1 = boot_fence(vec[:, 1:2], w1t, 8, 4)
            _adh(f_w1, f0, sync=False, reason="ldw fence chain")
            f_shm = boot_fence(vec[0:HP, 1:2], shmt, 8, (4 * HP) // 8)
            _adh(f_shm, f0, sync=False, reason="ldw fence chain")
            ps_geo = fap(ps_all, 0 * FXX, [[1, FXX]])
            ps_nall = fap(ps_all, 1 * FXX, [[1, FXX]])
            ps_asq = fap(ps_all, 2 * FXX, [[1, FXX]])
            ps_var = fap(ps_all, 3 * FXX, [[1, FXX]])
            ps_wl = fap(ps_all, 4 * FXX, [[1, FXX]])
            ps_dot = [fap(ps_all, (5 + i) * FXX, [[1, FXX]]) for i in range(4)]

            for ci in range(NCHK):
                x0 = ci * CR
                xh = x0 // HP
                p0 = x0 % HP
                # chunk temps (tag-shared): layout (xl, s), xl = x - x0 + 1 in [0, CHR)
                tA = cp.tile([C, CHR * SLICE_W], dt, tag="tA")
                tB = cp.tile([C, CHR * SLICE_W], dt, tag="tB")
                tC = cp.tile([C, CHR * SLICE_W], dt, tag="tC")
                tD = cp.tile([C, CHR * SLICE_W], dt, tag="tD")
                hrel = cp.tile([32, CR * SLICE_W], dt, tag="hrel")

                def fa_ap(dx, dy, rows=(0, CHR), cols=(0, SLICE_W), parts=None):
                    # fA view at chunk-local (xl+dx, s+dy); xl -> fA row x0 + xl
                    off = (x0 + rows[0] + dx) * SLICE_W + cols[0] + dy
                    a = fA[:] if parts is None else fA[parts[0]:parts[1], :]
                    return fap(a, off, [[SLICE_W, rows[1] - rows[0]],
                                        [1, cols[1] - cols[0]]])

                def t_ap(t, dx=0, dy=0, rows=(0, CHR), cols=(0, SLICE_W), parts=None):
                    off = (rows[0] + dx) * SLICE_W + cols[0] + dy
                    a = t[:] if parts is None else t[parts[0]:parts[1], :]
                    return fap(a, off, [[SLICE_W, rows[1] - rows[0]],
                                        [1, cols[1] - cols[0]]])

                RIN = (0, CHR)          # all chunk rows (incl halo)
                RMID = (1, CHR - 1)     # chunk rows = x in [x0, x0+CR)
                CMID = (1, SLICE_W - 1)

                # The PE silicon pulls LDWEIGHTS ahead of in-flight MATMULs, so a
                # RAW wait attached to a Matmult does not protect its stationary
                # (weights) read. Before each stationary-dataset matmul run, issue a
                # tiny fence matmul whose MOVING operand spans the dataset: its wait
                # is evaluated by the NX in order, blocking later LDWEIGHTS.
                from concourse.bass import _add_dep_helper

                def fence_mm(data_t, parts=(0, C), flat=False):
                    a = data_t[parts[0]:parts[1], :]
                    if flat:  # fully-written tile: spread points everywhere
                        n = a.shape[1]
                        step = max(1, (n - 1) // 15)
                        cnt = min(16, 1 + (n - 1) // step)
                        rhs = fap(a, 0, [[step, cnt]])
                    else:  # sample the interior window rows 1.., cols 1..2
                        nrow = min(8, CHR - 2)
                        rstep = max(1, (CHR - 2) // nrow)
                        rhs = fap(a, SLICE_W + 1,
                                  [[SLICE_W * rstep, nrow], [1, 2]])
                        cnt = nrow * 2
                    lhsT = vec[parts[0]:parts[1], 1:2]
                    outap = fap(ps_all[0:1, :], 9 * FXX, [[1, cnt]])
                    mm = nc.tensor.matmul(outap, lhsT, rhs, start=True, stop=True)
                    return mm.ins

                def stat_mm(group, data_t, c, rhs_ap, parts=(0, C), ncol=1,
                            halo_off=SLICE_W, fence=None):
                    lhsT = fap(data_t[parts[0]:parts[1], :],
                               halo_off + c + 1, [[SLICE_W, CR]])
                    base = ps_all[p0:p0 + CR, :]
                    outap = AP(base.tensor, base.offset + group * FXX + c * XH + xh,
                               [base.ap[0], [FXX, ncol]])
                    mm = nc.tensor.matmul(outap, lhsT, rhs_ap, start=True, stop=True)
                    if fence is not None:
                        _add_dep_helper(mm.ins, fence, sync=False,
                                        reason="ldweights raw fence")

                # ---- sobel: gx = smooth_x(f) diff_y ; gy = smooth_y(f) diff_x
                nc.vector.tensor_tensor(t_ap(tA, rows=RMID), fa_ap(-1, 0, RMID),
                                        fa_ap(+1, 0, RMID), Alu.add)
                nc.vector.scalar_tensor_tensor(
                    t_ap(tA, rows=RMID), fa_ap(0, 0, RMID), 2.0,
                    t_ap(tA, rows=RMID), Alu.mult, Alu.add)  # sx in tA
                nc.vector.tensor_tensor(
                    t_ap(tB, rows=RMID, cols=CMID), t_ap(tA, 0, +1, RMID, CMID),
                    t_ap(tA, 0, -1, RMID, CMID), Alu.subtract)  # gx in tB
                nc.vector.tensor_tensor(t_ap(tA, cols=CMID), fa_ap(0, -1, RIN, CMID),
                                        fa_ap(0, +1, RIN, CMID), Alu.add)
                nc.vector.scalar_tensor_tensor(
                    t_ap(tA, cols=CMID), fa_ap(0, 0, RIN, CMID), 2.0,
                    t_ap(tA, cols=CMID), Alu.mult, Alu.add)  # sy in tA
                nc.vector.tensor_tensor(
                    t_ap(tC, rows=RMID, cols=CMID), t_ap(tA, +1, 0, RMID, CMID),
                    t_ap(tA, -1, 0, RMID, CMID), Alu.subtract)  # gy in tC
                nc.scalar.activation(
                    t_ap(tA, rows=RMID, cols=CMID), t_ap(tB, rows=RMID, cols=CMID),
                    Act.Square)  # gx^2 in tA
                nc.vector.tensor_tensor(
                    t_ap(tB, rows=RMID, cols=CMID), t_ap(tC, rows=RMID, cols=CMID),
                    t_ap(tC, rows=RMID, cols=CMID), Alu.mult)  # gy^2 in tB
                nc.vector.tensor_tensor(
                    t_ap(tA, rows=RMID, cols=CMID), t_ap(tA, rows=RMID, cols=CMID),
                    t_ap(tB, rows=RMID, cols=CMID), Alu.add)   # mag^2 in tA
                nc.scalar.activation(
                    t_ap(tA, rows=RMID, cols=CMID), t_ap(tA, rows=RMID, cols=CMID),
                    Act.Sqrt)  # mag in tA
                fng = fence_mm(tA)
                for c in range(FW):
                    stat_mm(0, tA, c, vec[:, 0:1], fence=fng)  # geo

                # ---- f^2 -> nall ----
                nc.scalar.activation(
                    t_ap(tA, rows=RMID, cols=CMID), fa_ap(0, 0, RMID, CMID), Act.Square)
                fnn = fence_mm(tA)
                for c in range(FW):
                    stat_mm(1, tA, c, vec[:, 1:2], fence=fnn)

                # ---- pools on hf (parts 64:128) + absorption diff (parts 0:64)
                # end state: tD[0:64] = diff^2 ; tD[64:128] = poolsq/9 - pool^2/81
                P64 = (64, 128)
                P0_64 = (0, 64)
                nc.scalar.activation(t_ap(tB, parts=P64), fa_ap(0, 0, parts=P64),
                                     Act.Square)  # hf^2 (tB)
                nc.vector.tensor_tensor(t_ap(tC, cols=CMID, parts=P64),
                                        fa_ap(0, -1, RIN, CMID, P64),
                                        fa_ap(0, +1, RIN, CMID, P64), Alu.add)
                nc.vector.scalar_tensor_tensor(
                    t_ap(tC, cols=CMID, parts=P64), fa_ap(0, 0, RIN, CMID, P64), 1.0,
                    t_ap(tC, cols=CMID, parts=P64), Alu.mult, Alu.add)  # p3y(hf)
                nc.vector.tensor_tensor(
                    t_ap(tD, 0, 0, RMID, CMID, P64),
                    t_ap(tC, +1, 0, RMID, CMID, P64),
                    t_ap(tC, -1, 0, RMID, CMID, P64), Alu.add)
                nc.vector.scalar_tensor_tensor(
                    t_ap(tA, 0, 0, RMID, CMID, P64), t_ap(tC, 0, 0, RMID, CMID, P64),
                    1.0, t_ap(tD, 0, 0, RMID, CMID, P64), Alu.mult, Alu.add)  # pool(hf)
                nc.vector.tensor_tensor(t_ap(tC, cols=CMID, parts=P64),
                                        t_ap(tB, 0, -1, RIN, CMID, P64),
                                        t_ap(tB, 0, +1, RIN, CMID, P64), Alu.add)
                nc.vector.scalar_tensor_tensor(
                    t_ap(tC, cols=CMID, parts=P64), t_ap(tB, 0, 0, RIN, CMID, P64),
                    1.0, t_ap(tC, cols=CMID, parts=P64), Alu.mult, Alu.add)
                nc.vector.tensor_tensor(
                    t_ap(tD, 0, 0, RMID, CMID, P64), t_ap(tC, +1, 0, RMID, CMID, P64),
                    t_ap(tC, -1, 0, RMID, CMID, P64), Alu.add)
                nc.vector.scalar_tensor_tensor(
                    t_ap(tB, 0, 0, RMID, CMID, P64), t_ap(tC, 0, 0, RMID, CMID, P64),
                    1.0, t_ap(tD, 0, 0, RMID, CMID, P64), Alu.mult, Alu.add)  # poolsq
                # pool(hf)^2/81 -> tA (Square with scale 1/9)
                nc.scalar.activation(t_ap(tA, 0, 0, RMID, CMID, P64),
                                     t_ap(tA, 0, 0, RMID, CMID, P64), Act.Square,
                                     scale=1.0 / 9.0)
                # vdiff = poolsq/9 - pool^2/81 -> tD[64:128]
                nc.vector.scalar_tensor_tensor(
                    t_ap(tD, 0, 0, RMID, CMID, P64), t_ap(tB, 0, 0, RMID, CMID, P64),
                    1.0 / 9.0, t_ap(tA, 0, 0, RMID, CMID, P64),
                    Alu.mult, Alu.subtract)
                # diff^2 -> tD[0:64]
                nc.vector.tensor_scalar(
                    t_ap(tD, 0, 0, RMID, CMID, P0_64), fa_ap(0, 0, RMID, CMID, P0_64),
                    elft[:, 0:1], None, Alu.subtract)
                nc.scalar.activation(t_ap(tD, 0, 0, RMID, CMID, P0_64),
                                     t_ap(tD, 0, 0, RMID, CMID, P0_64), Act.Square)
                fnd = fence_mm(tD)
                for c in range(FW):
                    stat_mm(2, tD, c, vec[:, 2:4], ncol=2, fence=fnd)  # asq | var

                # ---- cost dots: f . f_shift for DIRS[0..3] ----
                for i in range(4):
                    dx, dy = DIRS[i]
                    nc.vector.tensor_tensor(
                        t_ap(tA, 0, 0, RMID, CMID), fa_ap(0, 0, RMID, CMID),
                        fa_ap(dx, dy, RMID, CMID), Alu.mult)
                    fni = fence_mm(tA)
                    for c in range(FW):
                        stat_mm(5 + i, tA, c, vec[:, 1:2], fence=fni)

                # ---- MLP: h = relu(w1.T f + b1) ; wl = w2.T h ----
                row0 = (x0 + 1) * SLICE_W
                total = CR * SLICE_W
                off = 0
                while off < total:
                    n = min(512, total - off)
                    psh = psbf.tile([32, 512], dt, tag="psh")
                    mmh = nc.tensor.matmul(
                        psh[:, 0:n], w1t[:],
                        fap(fA[:], row0 + off, [[1, n]]), start=True, stop=True)
                    _adh(mmh.ins, f_w1, sync=False, reason="w1 ldw fence")
                    nc.scalar.activation(
                        fap(hrel[:], off, [[1, n]]), psh[:, 0:n], Act.Relu,
                        bias=b1t[:, 0:1])
                    off += n
                fnh = fence_mm(hrel, parts=(0, 32), flat=True)
                for c in range(FW):
                    stat_mm(4, hrel, c, w2t[:], parts=(0, 32), halo_off=0, fence=fnh)

            # ---------------- evacuate psums to field tiles ----------------
            for ps, f in ((ps_geo, geo_f), (ps_nall, nall_f), (ps_var, var_f),
                          (ps_asq, asq_f), (ps_wl, wl_f),
                          (ps_dot[0], dot_f[0]), (ps_dot[1], dot_f[1]),
                          (ps_dot[2], dot_f[2]), (ps_dot[3], dot_f[3])):
                nc.scalar.copy(f[:], ps[:])

            # ---------------- cost assembly ----------------
            # rs = rsqrt(max(nall, 1e-24))
            rs_f = sp.tile([HP, FXX], dt, tag="rs")
            nc.scalar.activation(rs_f[:], nall_f[:], Act.Sqrt)
            nc.vector.tensor_scalar_max(rs_f[:], rs_f[:], 1e-12)
            nc.vector.reciprocal(rs_f[:], rs_f[:])
            # stage rs (padded col-major staging, pads = BIG irrelevant for rs: use 0)
            nc.sync.dma_start(fap(stg_rs[:].squeeze(0).unsqueeze(0), 0, [[1, STG]]),
                              fap(stg_rs[:], 0, [[1, STG]])) if False else None

            stgw = STG // HP
            bigt = sp.tile([HP, stgw], dt, tag="bigt")
            nc.vector.memset(bigt[:], BIGF)
            for s_t in stg + [stg_rs]:
                nc.sync.dma_start(
                    AP(s_t[:].tensor, 0, [[stgw, HP], [1, stgw]]), bigt[:])

            def field_to_stg(field_ap, stg_t):
                # field [HP, (c,xh)] -> stg[(c+2)*XPS + (x+2)], x = xh*HP + p
                for xh in range(XH):
                    dst = AP(stg_t[:].tensor, 2 * XPS + 2 + xh * HP,
                             [[1, HP], [XPS, FW]])
                    nc.sync.dma_start(dst, fap(field_ap, xh, [[XH, FW]]))

            def stg_to_field(stg_t, field_t, dx, dy):
                # field[x, c] <- stg at (c+dy, x+dx)
                for xh in range(XH):
                    src = AP(stg_t[:].tensor,
                             (2 + dy) * XPS + 2 + dx + xh * HP,
                             [[1, HP], [XPS, FW]])
                    nc.sync.dma_start(fap(field_t[:], xh, [[XH, FW]]), src)

            field_to_stg(rs_f[:], stg_rs)
            rsh_f = []
            for i in range(4):
                rfi = sp.tile([HP, FXX], dt, tag=f"rsh{i}")
                rsh_f.append(rfi)
            for i in range(4):
                dx, dy = DIRS[i]
                stg_to_field(stg_rs, rsh_f[i], dx, dy)

            cost_f = []
            for i in range(8):
                cfi = sp.tile([HP, FXX], dt, tag=f"cost{i}")
                cost_f.append(cfi)
            for i in range(4):
                # cost_i = max(1 - dot*rs*rs_sh, mask_i)
                t = cost_f[i]
                nc.vector.tensor_tensor(t[:], dot_f[i][:], rs_f[:], Alu.mult)
                nc.vector.tensor_tensor(t[:], t[:], rsh_f[i][:], Alu.mult)
                nc.vector.tensor_scalar(t[:], t[:], -1.0, 1.0, Alu.mult, Alu.add)
                nc.vector.tensor_tensor(
                    t[:], t[:], fap(mskt[:], i * FXX, [[1, FXX]]), Alu.max)
                field_to_stg(t[:], stg[i])
            for j in range(4, 8):
                # cost_j[u] = cost_{7-j}[u + dir_j]  (sim symmetry)
                i = 7 - j
                dxj, dyj = DIRS[j]
                stg_to_field(stg[i], cost_f[j], dxj, dyj)

            # ---------------- heuristic assembly ----------------
            om_f = sp.tile([HP, FXX], dt, tag="om")
            nc.scalar.activation(om_f[:], wl_f[:], Act.Sigmoid, bias=hsct[:, 4:5])
            absb_f = sp.tile([HP, FXX], dt, tag="absb")
            nc.vector.tensor_scalar_max(absb_f[:], asq_f[:], 0.0)
            nc.scalar.activation(absb_f[:], absb_f[:], Act.Sqrt)
            scat_f = sp.tile([HP, FXX], dt, tag="scat")
            nc.vector.tensor_scalar(scat_f[:], var_f[:], hsct[:, 3:4], -1.0,
                                    Alu.subtract, Alu.mult)
            h1 = sp.tile([HP, FXX], dt, tag="h1")
            h2 = sp.tile([HP, FXX], dt, tag="h2")
            heur_f = sp.tile([HP, FXX], dt, tag="heur")
            nc.vector.tensor_tensor(h1[:], om_f[:], scat_f[:], Alu.mult)   # om*scat
            nc.vector.tensor_tensor(h2[:], om_f[:], absb_f[:], Alu.mult)   # om*absorb
            nc.vector.tensor_tensor(h2[:], absb_f[:], h2[:], Alu.subtract)  # (1-om)abs
            nc.vector.tensor_scalar(heur_f[:], geo_f[:], hsct[:, 0:1], None, Alu.mult)
            nc.vector.scalar_tensor_tensor(h1[:], h1[:], hsct[:, 1:2], heur_f[:],
                                           Alu.mult, Alu.add)
            nc.vector.scalar_tensor_tensor(h1[:], h2[:], hsct[:, 2:3], h1[:],
                                           Alu.mult, Alu.add)
            nc.vector.tensor_scalar_max(heur_f[:], h1[:], 0.0)

            # ---------------- output slab (channels 0..8) ----------------
            slab = sp.tile([HP, BAND * XH * 10], dt, tag="slab")
            CO0 = T + 1  # first owned col in F_dom index

            def to_slab(field_t, ch):
                src = fap(field_t[:], CO0 * XH, [[XH, BAND], [1, XH]])
                dst = fap(slab[:], ch, [[XH * 10, BAND], [10, XH]])
                nc.vector.tensor_copy(dst, src)

            to_slab(heur_f, 0)
            for i in range(8):
                to_slab(cost_f[i], 1 + i)

            if dbg_names:
                for n, f_t in (("e_nall", nall_f), ("e_dot1", dot_f[1]),
                               ("e_cost1", cost_f[1]), ("e_rs", rs_f)):
                    nc.sync.dma_start(dbg_t[n][:], f_t[:])
                nc.sync.dma_start(dbg_t["d_fa"][:], fA[:])

            # ---------------- Bellman-Ford ----------------
            e_t = pp.tile([HP, 9 * YLEN], dt, tag="e")
            tmp_t = pp.tile([HP, 16 * YLEN], dt, tag="tmp")
            bufs = []
            for i in range(2):
                dbi = pp.tile([HP, 3 * SW], dt, tag=f"dbuf{i}")
                bufs.append(dbi)
            contrib = sp.tile([HP, 4 * T], dt, tag="contrib")

            nc.vector.memset(e_t[:], BIGF)
            # self candidates (g=1, bdir=1): e = 0 for all y
            nc.vector.memset(fap(e_t[:], 4 * YLEN, [[1, YLEN]]), 0.0)
            # e DMAs: for (g, bdir) != self: read staging of dir d at u
            for g, dxu in ((0, +1), (1, 0), (2, -1)):
                for bdir in range(3):
                    if g == 1 and bdir == 1:
                        continue
                    dvec = (-dxu, 1 - bdir)
                    d_idx = DIRS.index(dvec)
                    if d_idx <= 3:
                        s_t, adx, ady = stg[d_idx], 0, 0
                    else:
                        i = 7 - d_idx
                        s_t = stg[i]
                        adx, ady = DIRS[d_idx][0], DIRS[d_idx][1]
                    # u = (x + dxu, c_u = h + bdir); read stg at (c_u+ady, x+dxu+adx)
                    off = (bdir + ady + 2) * XPS + dxu + adx + 2
                    for b in range(XH):
                        srcb = AP(s_t[:].tensor, off + HP * b,
                                  [[1, HP], [XPS, HELD]])
                        dstb = fap(e_t[:], (g * 3 + bdir) * YLEN + 1 + YB * b,
                                   [[1, HELD]])
                        nc.sync.dma_start(dstb, srcb)

            for b in bufs:
                nc.vector.memset(b[:], BIGF)
            nc.sync.dma_start(fap(bufs[0][:], SW, [[1, SW]]), dinit_in[:])

            su_m = fap(shmt[:], 0 * HP, [[1, HP]])
            useam_m = fap(shmt[:], 1 * HP, [[1, HP]])
            sd_m = fap(shmt[:], 2 * HP, [[1, HP]])
            dseam_m = fap(shmt[:], 3 * HP, [[1, HP]])

            if NC > 1:
                pid = nc.sync.partition_id()
                jm_off = ((pid + NC - 1) & (NC - 1)) * (HP * 4 * T)
                jp_off = ((pid + 1) & (NC - 1)) * (HP * 4 * T)

            for k in range(K):
                cur, nxt = bufs[k % 2], bufs[(k + 1) % 2]
                if NC > 1 and k > 0 and k % T == 0:
                    # ---- halo exchange on cur.d ----
                    src = fap(cur[:], SW + 2 + T,
                              [[YB, 2], [BAND - T, 2], [1, T]])
                    nc.vector.tensor_copy(
                        fap(contrib[:], 0, [[2 * T, 2], [T, 2], [1, T]]), src)
                    nc.sync.dma_start(bounce_in[:], contrib[:])
                    nc.gpsimd.collective_compute(
                        "AllGather", mybir.AluOpType.bypass,
                        ins=[bounce_in[:]], outs=[bounce_out[:]],
                        replica_groups=[list(range(NC))])
                    # left halo <- (j-1).right strips: cols (b, lr=1, :)
                    lsrc = AP(bounce_out[:].tensor, jm_off + T,
                              [[4 * T, HP], [2 * T, 2], [1, T]])
                    nc.sync.dma_start(
                        fap(cur[:], SW + 2, [[YB, 2], [1, T]]), lsrc)
                    rsrc = AP(bounce_out[:].tensor, jp_off + 0,
                              [[4 * T, HP], [2 * T, 2], [1, T]])
                    nc.sync.dma_start(
                        fap(cur[:], SW + 2 + BAND + T, [[YB, 2], [1, T]]), rsrc)

                # ---- shifts: psum_du = Su . d (+ seam), psum_ds = Sd . d (+ seam)
                pdu = psbf.tile([HP, YLEN], dt, tag="pdu")
                pds = psbf.tile([HP, YLEN], dt, tag="pds")
                dcur = fap(cur[:], SW + 1, [[1, YLEN]])
                m1 = nc.tensor.matmul(pdu[:], su_m, dcur, start=True, stop=False)
                m2 = nc.tensor.matmul(fap(pdu[:], 0, [[1, YB]]), useam_m,
                                      fap(cur[:], SW + 1 + YB, [[1, YB]]),
                                      start=False, stop=True)
                m3 = nc.tensor.matmul(pds[:], sd_m, dcur, start=True, stop=False)
                m4 = nc.tensor.matmul(fap(pds[:], YB, [[1, YB]]), dseam_m,
                                      fap(cur[:], SW + 1, [[1, YB]]),
                                      start=False, stop=True)
                if k == 0:
                    for m in (m1, m2, m3, m4):
                        _adh(m.ins, f_shm, sync=False, reason="shm ldw fence")
                nc.scalar.copy(fap(cur[:], 1, [[1, YLEN]]), pdu[:])
                nc.scalar.copy(fap(cur[:], 2 * SW + 1, [[1, YLEN]]), pds[:])

                # ---- packed add + segmented reduce-min ----
                nc.vector.tensor_tensor(
                    fap(tmp_t[:], 0, [[3, 3], [1, 3], [16, YLEN]]),
                    fap(cur[:], 0, [[SW, 3], [1, 3], [1, YLEN]]),
                    fap(e_t[:], 0, [[3 * YLEN, 3], [YLEN, 3], [1, YLEN]]),
                    Alu.add)
                nc.vector.tensor_reduce(
                    fap(nxt[:], SW + 1, [[1, YLEN]]),
                    fap(tmp_t[:], 0, [[16, YLEN], [1, 9]]),
                    axis=mybir.AxisListType.X, op=Alu.min)

            # ---- dist -> slab channel 9 ----
            fin = bufs[K % 2]
            # slab idx = (co*XH + xh)*10 + 9 ; src (b=xh, h=T+co)
            src = fap(fin[:], SW + 2 + T, [[YB, 2], [1, BAND]])
            dst = fap(slab[:], 9, [[10, XH], [XH * 10, BAND]])
            nc.vector.tensor_copy(dst, src)

            if dbg_names:
                for n, f_t in (("d_nall", nall_f), ("d_rs", rs_f),
                               ("d_dot0", dot_f[0]), ("d_dot1", dot_f[1]),
                               ("d_cost0", cost_f[0]), ("d_cost1", cost_f[1]),
                               ("d_geo", geo_f), ("d_wl", wl_f),
                               ("d_rsh0", rsh_f[0]), ("d_rsh1", rsh_f[1]),
                               ("d_rsh2", rsh_f[2]), ("d_rsh3", rsh_f[3])):
                    nc.sync.dma_start(dbg_t[n][:], f_t[:])
            nc.sync.dma_start(out_t[:], slab[:])

    nc.compile()
    return nc


# ---------------------------------------------------------------- host ------

def softplus32(x):
    x = np.float32(x)
    return F32(np.log1p(np.exp(np.float64(x))))


def host_prepare(cfg, features, delta, gamma, beta, w1, b1, w2, b2,
                 start_node, end_node):
    H, W, C, NC, T = (cfg[k] for k in ("H", "W", "C", "NC", "T"))
    HP, BAND, HELD, SLICE_W, FW, XP, YB, SW, XH = (
        cfg[k] for k in ("HP", "BAND", "HELD", "SLICE_W", "FW", "XP", "YB", "SW",
                         "XH"))
    features = np.asarray(features, F32)
    w1 = np.asarray(w1, F32).reshape(C, 32)
    b1 = np.asarray(b1, F32).reshape(32, 1)
    w2 = np.asarray(w2, F32).reshape(32, 1)
    b2 = F32(np.asarray(b2).reshape(()))
    sx_, sy_ = [int(v) for v in np.asarray(start_node).ravel()]
    ex_, ey_ = [int(v) for v in np.asarray(end_node).ravel()]

    d_soft, g_soft, b_soft = softplus32(delta), softplus32(gamma), softplus32(beta)

    # info_goal_hf = var_hf at end node (3x3 window, zero pad, count_include_pad)
    hf = features[:, :, C // 2:]
    x0e, x1e = max(0, ex_ - 1), min(H, ex_ + 2)
    y0e, y1e = max(0, ey_ - 1), min(W, ey_ + 2)
    win = hf[x0e:x1e, y0e:y1e, :].astype(F32)
    s1 = win.sum(axis=(0, 1), dtype=F32) / F32(9.0)
    s2 = (win * win).sum(axis=(0, 1), dtype=F32) / F32(9.0)
    info_goal = F32((s2 - s1 * s1).sum(dtype=F32))
    elf = features[ex_, ey_, :C // 2].astype(F32).reshape(64, 1)

    # shift matrices
    su = np.zeros((HP, HP), F32)
    sd = np.zeros((HP, HP), F32)
    for i in range(HP - 1):
        su[i + 1, i] = 1.0   # du[i] = d[i+1]
        sd[i, i + 1] = 1.0   # ds[i] = d[i-1]
    useam = np.zeros((HP, HP), F32)
    useam[0, HP - 1] = 1.0   # du[HP-1] += d_blk1[0]
    dseam = np.zeros((HP, HP), F32)
    dseam[HP - 1, 0] = 1.0   # ds[0] += d_blk0[HP-1]
    shm = np.concatenate([su, useam, sd, dseam], axis=1)

    hsc = np.zeros((HP, 8), F32)
    hsc[:, 0] = d_soft
    hsc[:, 1] = g_soft
    hsc[:, 2] = b_soft
    hsc[:, 3] = info_goal
    hsc[:, 4] = b2

    in_maps = []
    for j in range(NC):
        SC0 = j * BAND - (T + 2)
        fa = np.zeros((XP, SLICE_W, C), F32)
        s0 = max(0, -SC0)
        s1_ = min(SLICE_W, W - SC0)
        fa[1:H + 1, s0:s1_, :] = features[:, SC0 + s0:SC0 + s1_, :]
        fa = np.ascontiguousarray(fa.transpose(2, 0, 1).reshape(C, -1))

        msk = np.zeros((HP, 4, FW, XH), F32)
        xs = (np.arange(HP)[:, None, None, None] +
              HP * np.arange(XH)[None, None, None, :])
        cs = np.arange(FW)[None, None, :, None]
        gc = j * BAND - (T + 1) + cs
        for i in range(4):
            dx, dy = DIRS[i]
            invalid = ((gc < 0) | (gc >= W) | (gc + dy < 0) | (gc + dy >= W) |
                       (xs + dx < 0) | (xs + dx >= H))
            msk[:, i, :, :] = np.where(invalid, BIGF, 0.0)[:, 0]
        msk = msk.reshape(HP, -1)

        dinit = np.full((HP, SW), BIGF, F32)
        hband0 = j * BAND - T
        hh = sy_ - hband0
        if 0 <= hh < HELD:
            dinit[sx_ % HP, 2 + YB * (sx_ // HP) + hh] = 0.0

        in_maps.append({
            "fa": fa, "w1": w1, "b1": b1, "w2": w2, "elf": elf,
            "hsc": hsc, "msk": msk.astype(F32), "dinit": dinit, "shm": shm,
        })
    return in_maps


def host_assemble(cfg, results):
    H, W, NC = cfg["H"], cfg["W"], cfg["NC"]
    HP, BAND, XH = cfg["HP"], cfg["BAND"], cfg["XH"]
    out = np.zeros((H, W, 10), F32)
    for j in range(NC):
        slab = results[j]["out"].reshape(HP, BAND, XH, 10)
        # slab[p, co, xh, ch] -> x = xh*HP + p, col = j*BAND + co
        blk = slab.transpose(2, 0, 1, 3).reshape(H, BAND, 10)
        out[:, j * BAND:(j + 1) * BAND, :] = blk
    return out


_PROG_CACHE = {}


def get_program(cfg):
    key = tuple(sorted((k, v) for k, v in cfg.items()))
    if key not in _PROG_CACHE:
        _PROG_CACHE[key] = build_program(cfg)
    return _PROG_CACHE[key]


def kernel(**inputs):
    cfg = make_cfg()
    nc = get_program(cfg)
    in_maps = host_prepare(cfg, **inputs)
    from concourse.bass_utils import run_bass_kernel_spmd
    res = run_bass_kernel_spmd(nc, in_maps, core_ids=list(range(cfg["NC"])))
    return host_assemble(cfg, res.results)

